# revision 1
# baseline (speedup 1.0000x reference)
"""Trainium2 Bass kernel for a dense transformer block (nn_Block_83880711291003).

Full (unsharded) inputs in, full output out. 8 NeuronCores:
  core c -> batch b = c//2, parity p = c%2. Parity 0 owns q-regions {0,3}
  (512 tokens each), parity 1 owns {1,2}. Two compiled programs (one per
  parity) with EXACT causal kv spans per slot: p0 -> (4,16) kv tiles,
  p1 -> (8,12); both total 20 kv-tile iterations per head, so the cores
  are balanced and no padded attention work is done.

All heavy matmuls run in fp8 e4m3 with DoubleRow perf mode (2 contraction
subtiles per instruction, 0.5 PE cycles/row). Weights are pre-scaled by 64
on the host so every PSUM evacuation is a plain copy: the combined scale
folds into the softmax exp scale (qk: 1/64^2), the projection weights
(wp/64), and the final FFN residual add (1/64^2).
"""

import sys

for _p in ("/opt/trn_rl_repo", "/root/.axon_site/_ro/trn_rl_repo"):
    if _p not in sys.path:
        sys.path.append(_p)

from contextlib import ExitStack

import ml_dtypes
import numpy as np

import concourse.bass as bass
import concourse.tile as tile
from concourse import mybir
from concourse.bass_utils import run_bass_kernel_spmd
from concourse.masks import make_identity
from concourse.vector_clock import ScopedClock

FP32 = mybir.dt.float32
BF16 = mybir.dt.bfloat16
FP8 = mybir.dt.float8e4
BFNP = ml_dtypes.bfloat16
E4NP = ml_dtypes.float8_e4m3
DR = mybir.MatmulPerfMode.DoubleRow

B, T, D = 4, 2048, 512
H, DK = 8, 64
OWN_T = T // 2         # 1024 own query tokens per core
FF = 4 * D             # 2048
EPS = 1e-5
WSCALE = 64.0          # host-side fp8 weight scale
EXP_SCALE = 1.0 / (8.0 * WSCALE * WSCALE)   # 1/sqrt(dk) and q,k descale
REGIONS = ((0, 3), (1, 2))  # q-region (512-token) assignment per parity

ACT = mybir.ActivationFunctionType

# ---------------------------------------------------------------------------
# Workaround: this walrus build rejects >1 semaphore wait per instruction.
# ---------------------------------------------------------------------------
_uid = [0]


def _split_multi_waits(nc):
    for blk in nc.m.functions[0].blocks:
        insts = list(blk.instructions)
        out, changed = [], False
        for inst in insts:
            si = inst.sync_info
            waits = list(si.on_wait) if si else []
            if len(waits) > 1:
                changed = True
                for w in waits[:-1]:
                    _uid[0] += 1
                    nop = mybir.InstNoOp(name=f"I-waitfix-{_uid[0]}", ins=[], outs=[])
                    nop.engine = inst.engine
                    nop.sync_info = mybir.SyncInfo(on_wait=[w], on_update=[])
                    out.append(nop)
                inst.sync_info = mybir.SyncInfo(
                    on_wait=[waits[-1]], on_update=list(si.on_update)
                )
            out.append(inst)
        if changed:
            blk.instructions = out


def _patched_drain_and_barrier(self, tick_clock, wait_clock):
    nc = self.nc
    probe = nc.sync.nop()
    wait_clock.add_sem_waits(probe.ins, ScopedClock({None: tick_clock.global_clock}))
    nc.sync.drain()
    nc.all_engine_barrier()
    popped = nc._tile_sem_poison_stack.pop()
    assert popped is self._sem_poison
    nc.clear_and_free_semaphores(list(self.sems.allocated().values()))
    nc.all_engine_barrier()


tile.TileContext._drain_and_barrier = _patched_drain_and_barrier


# ---------------------------------------------------------------------------
# Device program (one per parity)
# ---------------------------------------------------------------------------
def _build_program(parity):
    r0, r1 = REGIONS[parity]
    nc = bass.Bass("TRN2", target_bir_lowering=False, debug=False)

    din = {}
    for name, shape, dt in [
        ("x_full", [T, D], FP32),
        ("x_own", [OWN_T, D], FP32),
        ("wq", [2, 128, 4, 2, 128], FP8),
        ("wk", [2, 128, 4, 2, 128], FP8),
        ("wv", [2, 128, 2, 512], FP8),
        ("wp", [H, 64, D], BF16),
        ("w1", [2, 128, 16, 2, 128], FP8),
        ("w2", [8, 128, 2, D], FP8),
        ("mask_tri", [128, 128], FP8),
    ]:
        din[name] = nc.dram_tensor(name, shape, dt, kind="ExternalInput").ap()
    out_dram = nc.dram_tensor("out", [OWN_T, D], FP32, kind="ExternalOutput").ap()
    rb_dram = nc.dram_tensor("rb_scratch", [8, OWN_T], BF16, kind="Internal").ap()

    with tile.TileContext(nc) as tc, ExitStack() as ctx, \
            nc.allow_low_precision(reason="fp8/bf16 pipeline validated offline"):
        P = ctx.enter_context

        persist = P(tc.tile_pool(name="persist", bufs=1))
        wpool = P(tc.tile_pool(name="weights", bufs=1))
        xio = P(tc.tile_pool(name="xio", bufs=16))
        small = P(tc.tile_pool(name="small", bufs=6))
        hpool = P(tc.tile_pool(name="htok", bufs=4))
        ptpool = P(tc.tile_pool(name="pt", bufs=6))
        rbpool = P(tc.tile_pool(name="rbb", bufs=3))
        opool = P(tc.tile_pool(name="outio", bufs=3))
        psPair = P(tc.tile_pool(name="psPair", bufs=2, space="PSUM"))  # 2-bank
        psB = P(tc.tile_pool(name="psB", bufs=2, space="PSUM"))        # 1-bank

        # ---- DMA order: first x tiles, QKV weights, rest of x, the rest ----
        x_pre = []
        for t in range(16):
            x_t = xio.tile([128, D], FP32, tag="xin", name="xin")
            x_pre.append(x_t)
        for t in range(4):
            nc.sync.dma_start(x_pre[t][:], din["x_full"][t * 128:(t + 1) * 128, :])
        wq_t = [wpool.tile([128, 4, 2, 128], FP8, tag=f"wq{p}", name=f"wq{p}") for p in range(2)]
        wk_t = [wpool.tile([128, 4, 2, 128], FP8, tag=f"wk{p}", name=f"wk{p}") for p in range(2)]
        wv_t = [wpool.tile([128, 2, 512], FP8, tag=f"wv{p}", name=f"wv{p}") for p in range(2)]
        for p in range(2):
            nc.sync.dma_start(wq_t[p][:], din["wq"][p])
            nc.sync.dma_start(wk_t[p][:], din["wk"][p])
            nc.sync.dma_start(wv_t[p][:], din["wv"][p])
        for t in range(4, 16):
            nc.sync.dma_start(x_pre[t][:], din["x_full"][t * 128:(t + 1) * 128, :])
        wp_t = [wpool.tile([64, D], BF16, tag=f"wp{h}", name=f"wp{h}") for h in range(H)]
        for h in range(H):
            nc.sync.dma_start(wp_t[h][:], din["wp"][h])
        mask_tri = wpool.tile([128, 128], FP8, tag="mtri", name="mtri")
        nc.sync.dma_start(mask_tri[:], din["mask_tri"][:])
        w1_t = [wpool.tile([128, 16, 2, 128], FP8, tag=f"w1{p}", name=f"w1{p}") for p in range(2)]
        w2_t = [wpool.tile([128, 2, D], FP8, tag=f"w2{u}", name=f"w2{u}") for u in range(8)]
        for p in range(2):
            nc.sync.dma_start(w1_t[p][:], din["w1"][p])
        for u in range(8):
            nc.sync.dma_start(w2_t[u][:], din["w2"][u])

        ident8 = wpool.tile([128, 128], BF16, tag="id8", name="id8")
        make_identity(nc, ident8[:])
        eps_t = wpool.tile([128, 1], FP32, tag="eps", name="eps")
        nc.vector.memset(eps_t[:], EPS)

        # ---- persistent activations ----
        hT = persist.tile([128, 2, 2, 16, 128], FP8, tag="hT", name="hT")
        kT = [persist.tile([128, 2, 16, 128], FP8, tag=f"kT{g}", name=f"kT{g}") for g in range(2)]
        qT = [persist.tile([128, 2, 8, 128], FP8, tag=f"qT{g}", name=f"qT{g}") for g in range(2)]
        v1 = persist.tile([128, 8, H, 2, 80], FP8, tag="v1", name="v1")
        oTu = [persist.tile([65, OWN_T], BF16, tag=f"oTu{h}", name=f"oTu{h}") for h in range(H)]
        den4 = persist.tile([4, 2, OWN_T], BF16, tag="den4", name="den4")
        x_o = [persist.tile([128, D], FP32, tag=f"xo{t}", name=f"xo{t}") for t in range(8)]
        x2 = [persist.tile([128, D], FP32, tag=f"x2_{t}", name=f"x2_{t}") for t in range(8)]
        h2T = persist.tile([128, 2, 2, 8, 128], FP8, tag="h2T", name="h2T")

        nc.vector.memset(v1[:, :, :, :, 64], 1.0)
        nc.vector.memset(v1[:, :, :, 0, 65:80], 0.0)
        nc.vector.memset(v1[:, :, :, 1, 65:80], 0.0)
        for t in range(8):
            nc.sync.dma_start(x_o[t][:], din["x_own"][t * 128:(t + 1) * 128, :])

        def layer_norm(x_t, h_out):
            """h_out (fp8) = (x_t - mean) / sqrt(var+eps)."""
            st = small.tile([128, 6], FP32, tag="bnst", name="bnst")
            nc.vector.bn_stats(out=st[:], in_=x_t[:])
            mv = small.tile([128, 2], FP32, tag="bnmv", name="bnmv")
            nc.vector.bn_aggr(out=mv[:], in_=st[:])
            rs = small.tile([128, 1], FP32, tag="rs", name="rs")
            nc.scalar.activation(out=rs[:], in_=mv[:, 1:2], func=ACT.Sqrt,
                                 bias=eps_t[:], scale=1.0)
            nc.vector.reciprocal(out=rs[:], in_=rs[:])
            nc.gpsimd.tensor_scalar(
                out=h_out[:], in0=x_t[:], scalar1=mv[:, 0:1], scalar2=rs[:],
                op0=mybir.AluOpType.subtract, op1=mybir.AluOpType.mult)

        def transpose_pair_evac(dst, tp, psT, evac_eng):
            """dst[:, P, 2tp:2tp+2, :, :] <- 8 transposed [128,128] in psT.
            dst layout [128, 2P, blk, 2sub, 128]; psT layout (tt, c=2P+sub, u)."""
            for p in range(2):
                out_ap = dst[:, p, :, 2 * tp:2 * tp + 2, :]
                in_ap = bass.AP(
                    tensor=psT[:].tensor, offset=psT[:].offset + 256 * p,
                    ap=[psT[:].ap[0], [128, 2], [512, 2], [1, 128]])
                if evac_eng == "act2" or (evac_eng == "act") == (p == 0):
                    nc.scalar.copy(out_ap, in_ap)
                else:
                    nc.vector.tensor_copy(out_ap, in_ap)

        # ---- phase A building blocks ----
        def full_pair(tp, evac_eng="act"):
            """LN1 + transposes + V proj for x tiles 2tp, 2tp+1."""
            psT = psB.tile([128, 8, 128], BF16, tag="psT", name="psT")
            for tt in range(2):
                t = 2 * tp + tt
                x_t = x_pre[t]
                h_t = hpool.tile([128, D], BF16, tag="h1", name="h1")
                layer_norm(x_t, h_t)
                for c in range(4):
                    nc.tensor.transpose(psT[:, tt * 4 + c, :],
                                        h_t[:, c * 128:(c + 1) * 128], ident8[:])
            transpose_pair_evac(hT, tp, psT, evac_eng)

            for tt in range(2):
                t = 2 * tp + tt
                psV = psB.tile([128, 512], FP32, tag="one", name="psV")
                for p in range(2):
                    nc.tensor.matmul(psV[:],
                                     hT[:, p, :, t, :],
                                     wv_t[p][:], start=(p == 0), stop=(p == 1),
                                     perf_mode=DR)
                v_in = psV[:].rearrange("p (h k) -> p h k", h=H)
                if tt == 0:
                    nc.vector.tensor_copy(v1[:, tp, :, tt, 0:64], v_in)
                else:
                    nc.scalar.copy(v1[:, tp, :, tt, 0:64], v_in)

        def k_chunk(c, evac_eng):
            for g in range(2):
                for X in range(2):
                    psK = psB.tile([128, 512], FP32, tag="one", name="psK")
                    for p in range(2):
                        nc.tensor.matmul(
                            psK[:],
                            wk_t[p][:, 2 * g + X, :, :],
                            hT[:, p, :, 4 * c:4 * c + 4, :],
                            start=(p == 0), stop=(p == 1), perf_mode=DR)
                    kout = kT[g][:, X, 4 * c:4 * c + 4, :]
                    if (X == 0) == (evac_eng == "dve"):
                        nc.vector.tensor_copy(kout, psK[:])
                    else:
                        nc.scalar.copy(kout, psK[:])

        def q_chunk(s):
            r = (r0, r1)[s]
            for g in range(2):
                for X in range(2):
                    psQ = psB.tile([128, 512], FP32, tag="one", name="psQ")
                    for p in range(2):
                        nc.tensor.matmul(
                            psQ[:],
                            wq_t[p][:, 2 * g + X, :, :],
                            hT[:, p, :, 4 * r:4 * r + 4, :],
                            start=(p == 0), stop=(p == 1), perf_mode=DR)
                    nc.scalar.copy(qT[g][:, X, 4 * s:4 * s + 4, :], psQ[:])

        # ---- attention slot ----
        def att_head(s, h, filler=None, inline_epi=False):
            r = (r0, r1)[s]
            pairs = 2 * (r + 1)
            g, h4 = h // 4, h % 4
            krows = kT[g][32 * h4:32 * h4 + 32, :, :, :]
            qrows = qT[g][32 * h4:32 * h4 + 32, :, 4 * s:4 * s + 4, :]
            psPV = psB.tile([128, 512], FP32, tag="one", name="psPV")[0:80, :]
            for jp in range(pairs):
                dp = jp - (pairs - 2)   # >=0 on the 2 diagonal pairs
                c0 = 256 if dp == 1 else 0
                psS = psPair.tile([128, 2, 512], FP32, tag="pair", name="psS")
                b0 = c0 // 128
                for d in range(2):
                    j = 2 * jp + d
                    nc.tensor.matmul(psS[:, d, c0:512],
                                     krows[:, :, j, :],
                                     qrows[:, :, b0:4, :],
                                     start=True, stop=True, perf_mode=DR,
                                     tile_position=(32 * h4, 0))
                pT = ptpool.tile([128, 2, 512], FP8, tag="pt", name="pT")
                nc.scalar.activation(out=pT[:, :, c0:512], in_=psS[:, :, c0:512],
                                     func=ACT.Exp, scale=EXP_SCALE)
                if dp >= 0:
                    nc.gpsimd.memset(pT[:, 1, c0:c0 + 128], 0.0)
                    for d in range(2):
                        col = (2 * dp + d) * 128
                        nc.vector.tensor_mul(pT[:, d, col:col + 128],
                                             pT[:, d, col:col + 128],
                                             mask_tri[:])
                nc.tensor.matmul(psPV[:, c0:512], v1[:, jp, h, :, :],
                                 pT[:, :, c0:512],
                                 start=(jp == 0), stop=(jp == pairs - 1),
                                 perf_mode=DR, skip_group_check=True)
                if filler is not None and jp == min(2, pairs - 1):
                    filler()
                    filler = None
            if filler is not None:
                filler()
            nc.vector.tensor_copy(oTu[h][:, s * 512:(s + 1) * 512], psPV[0:65, :])
            if inline_epi:
                rs1 = rbpool.tile([1, 512], BF16, tag="rs1", name="rs1")
                nc.vector.reciprocal(out=rs1[:], in_=psPV[64:65, :])
                nc.sync.dma_start(rb_dram[h:h + 1, s * 512:(s + 1) * 512], rs1[:])
                rbb = rbpool.tile([64, 512], BF16, tag="rbb", name="rbb")
                srcd = rb_dram[h:h + 1, s * 512:(s + 1) * 512]
                bcast = bass.AP(tensor=srcd.tensor, offset=srcd.offset,
                                ap=[[0, 64], srcd.ap[1]])
                nc.sync.dma_start(rbb[:], bcast)
                nc.vector.tensor_mul(oTu[h][0:64, s * 512:(s + 1) * 512],
                                     oTu[h][0:64, s * 512:(s + 1) * 512], rbb[:])
            else:
                nc.sync.dma_start(
                    den4[h % 4:h % 4 + 1, h // 4, s * 512:(s + 1) * 512],
                    oTu[h][64:65, s * 512:(s + 1) * 512])

        # ---- epilogue: softmax normalization for a 4-head half of slot s ----
        def epi_prep(s, half=0):
            rb4 = rbpool.tile([4, 512], BF16, tag="rb4", name="rb4")
            nc.vector.reciprocal(out=rb4[:],
                                 in_=den4[0:4, half, s * 512:(s + 1) * 512])
            nc.sync.dma_start(rb_dram[4 * half:4 * half + 4,
                                      s * 512:(s + 1) * 512], rb4[:])
            for h in range(4 * half, 4 * half + 4):
                rbb = rbpool.tile([64, 512], BF16, tag="rbb", name="rbb")
                src = rb_dram[h:h + 1, s * 512:(s + 1) * 512]
                bcast = bass.AP(tensor=src.tensor, offset=src.offset,
                                ap=[[0, 64], src.ap[1]])
                nc.sync.dma_start(rbb[:], bcast)
                nc.vector.tensor_mul(oTu[h][0:64, s * 512:(s + 1) * 512],
                                     oTu[h][0:64, s * 512:(s + 1) * 512], rbb[:])

        # ---- epilogue: proj + LN2 + h2T for one own tile ----
        psT2_ref = [None]

        def epi_tile(s, tt, evac_eng):
            t = s * 4 + tt
            if tt % 2 == 0:
                psT2_ref[0] = psB.tile([128, 8, 128], BF16, tag="psT", name="psT2")
            psP = psB.tile([128, 512], FP32, tag="one", name="psP")
            for h in range(H):
                nc.tensor.matmul(psP[:], oTu[h][0:64, t * 128:(t + 1) * 128],
                                 wp_t[h][:], start=(h == 0), stop=(h == 7))
            nc.vector.tensor_add(x2[t][:], psP[:], x_o[t][:])
            h_t = hpool.tile([128, D], BF16, tag="h1", name="h2")
            layer_norm(x2[t], h_t)
            for c in range(4):
                nc.tensor.transpose(psT2_ref[0][:, (tt % 2) * 4 + c, :],
                                    h_t[:, c * 128:(c + 1) * 128], ident8[:])
            if tt % 2 == 1:
                transpose_pair_evac(h2T, (s * 4 + tt - 1) // 2, psT2_ref[0], evac_eng)

        # ---- FFN ----
        def ffn_chunk(c, relu_engs, use_pair=False):
            f1 = []
            for u in range(8):
                f1u = ptpool.tile([128, 4, 2, 128], FP8, tag="f1", name="f1", bufs=9)
                if use_pair:
                    psFp = psPair.tile([128, 2, 512], FP32, tag="pair", name="psFp")
                    for e in range(2):
                        for p in range(2):
                            nc.tensor.matmul(
                                psFp[:, e, :],
                                w1_t[p][:, 2 * u + e, :, :],
                                h2T[:, p, :, 4 * c:4 * c + 4, :],
                                start=(p == 0), stop=(p == 1), perf_mode=DR)
                    for e in range(2):
                        fo = f1u[:, :, e, :]
                        fi = psFp[:, e, :].rearrange("p (b u) -> p b u", b=4)
                        if relu_engs[u % len(relu_engs)] == "act":
                            nc.scalar.activation(out=fo, in_=fi,
                                                 func=ACT.Relu, scale=1.0)
                        else:
                            nc.vector.tensor_scalar_max(fo, fi, 0.0)
                    f1.append(f1u)
                    continue
                for e in range(2):
                    psF = psB.tile([128, 512], FP32, tag="one", name="psF")
                    for p in range(2):
                        nc.tensor.matmul(
                            psF[:],
                            w1_t[p][:, 2 * u + e, :, :],
                            h2T[:, p, :, 4 * c:4 * c + 4, :],
                            start=(p == 0), stop=(p == 1), perf_mode=DR)
                    fo = f1u[:, :, e, :]
                    fi = psF[:].rearrange("p (b u) -> p b u", b=4)
                    if relu_engs[(2 * u + e) % len(relu_engs)] == "act":
                        nc.scalar.activation(out=fo, in_=fi,
                                             func=ACT.Relu, scale=1.0)
                    else:
                        nc.vector.tensor_scalar_max(fo, fi, 0.0)
                f1.append(f1u)
            for tt in range(4):
                t = c * 4 + tt
                psO2 = psB.tile([128, 512], FP32, tag="one", name="psO2")
                for u in range(8):
                    nc.tensor.matmul(psO2[:], f1[u][:, tt, :, :],
                                     w2_t[u][:], start=(u == 0), stop=(u == 7),
                                     perf_mode=DR)
                o_t = opool.tile([128, D], FP32, tag="ot", name="ot")
                nc.vector.scalar_tensor_tensor(
                    out=o_t[:], in0=psO2[:], scalar=1.0 / (WSCALE * WSCALE),
                    in1=x2[t][:], op0=mybir.AluOpType.mult,
                    op1=mybir.AluOpType.add)
                nc.sync.dma_start(out_dram[t * 128:(t + 1) * 128, :], o_t[:])

        f1_split = {}

        def ffn_mm1_half(c, hf):
            relu_engs = ("act", "dve")
            for u in range(8):
                if hf == 0:
                    f1_split[u] = ptpool.tile([128, 4, 2, 128], FP8, tag="f1",
                                              name="f1", bufs=9)
                f1u = f1_split[u]
                psFp = psPair.tile([128, 2, 512], FP32, tag="pair", name="psFp")
                b0 = c * 4 + hf * 2
                for e in range(2):
                    for p in range(2):
                        nc.tensor.matmul(
                            psFp[:, e, hf * 256:hf * 256 + 256],
                            w1_t[p][:, 2 * u + e, :, :],
                            h2T[:, p, :, b0:b0 + 2, :],
                            start=(p == 0), stop=(p == 1), perf_mode=DR,
                            skip_group_check=True)
                for e in range(2):
                    fa = f1u[:, 2 * hf:2 * hf + 2, e, :]
                    pa = psFp[:, e, hf * 256:hf * 256 + 256].rearrange(
                        "p (b u) -> p b u", b=2)
                    if relu_engs[u % 2] == "act":
                        nc.scalar.activation(out=fa, in_=pa, func=ACT.Relu,
                                             scale=1.0)
                    else:
                        nc.vector.tensor_scalar_max(fa, pa, 0.0)

        def ffn_mm2_tile(c, tt):
            t = c * 4 + tt
            psO2 = psB.tile([128, 512], FP32, tag="one", name="psO2")
            for u in range(8):
                nc.tensor.matmul(psO2[:], f1_split[u][:, tt, :, :],
                                 w2_t[u][:], start=(u == 0), stop=(u == 7),
                                 perf_mode=DR)
            o_t = opool.tile([128, D], FP32, tag="ot", name="ot")
            nc.vector.scalar_tensor_tensor(
                out=o_t[:], in0=psO2[:], scalar=1.0 / (WSCALE * WSCALE),
                in1=x2[t][:], op0=mybir.AluOpType.mult,
                op1=mybir.AluOpType.add)
            nc.sync.dma_start(out_dram[t * 128:(t + 1) * 128, :], o_t[:])

        # ---- emission order: interleave attention with phase-A remainder,
        # epilogue tiles, and the first FFN chunk so no engine queue has
        # long head-of-line stalls.
        pairs_pre = 2 * (r0 + 1)
        for tp in range(pairs_pre):
            full_pair(tp, "act")
            if tp % 2 == 1:
                k_chunk(tp // 2, "dve" if tp % 4 == 1 else "pool")
        q_chunk(0)

        kv_pairs = 2 * (r1 + 1)   # kv pairs actually consumed by attention
        rem = []
        for tp in range(pairs_pre, kv_pairs):
            rem.append(("fp", tp))
            if tp % 2 == 1:
                rem.append(("kc", tp // 2))
        rem.append(("qc", 1))
        per_head = (len(rem) + H - 1) // H

        def mk_filler_a(units):
            def f():
                for unit in units:
                    if unit[0] == "fp":
                        full_pair(unit[1], "act")
                    elif unit[0] == "kc":
                        k_chunk(unit[1], "dve" if unit[1] % 2 == 0 else "pool")
                    else:
                        q_chunk(unit[1])
            return f

        for h in range(H):
            att_head(0, h, mk_filler_a(rem[h * per_head:(h + 1) * per_head]))

        def mk_filler_b(h):
            def f():
                if h == 1:
                    epi_prep(0, 0)
                elif h == 2:
                    epi_prep(0, 1)
                elif 3 <= h < 7:
                    epi_tile(0, h - 3, "act")
            return f

        for h in range(H):
            att_head(1, h, mk_filler_b(h), inline_epi=(h >= 4))
            if h == 3:
                epi_prep(1, 0)
        ffn_chunk(0, ("act", "dve"), use_pair=True)
        # FFN chunk 1 in token halves, interleaved with the epilogue tiles
        epi_tile(1, 0, "act")
        epi_tile(1, 1, "act")
        ffn_mm1_half(1, 0)
        epi_tile(1, 2, "act")
        ffn_mm2_tile(1, 0)
        epi_tile(1, 3, "act")
        ffn_mm2_tile(1, 1)
        ffn_mm1_half(1, 1)
        ffn_mm2_tile(1, 2)
        ffn_mm2_tile(1, 3)

    _split_multi_waits(nc)
    return nc


_NC_CACHE = {}


def _get_nc(parity):
    if parity not in _NC_CACHE:
        _NC_CACHE[parity] = _build_program(parity)
    return _NC_CACHE[parity]


# ---------------------------------------------------------------------------
# Host side
# ---------------------------------------------------------------------------
def _fold_weights(Wq, bq, Wk, bk, Wv, bv, Wp, bp, W1, b1, W2, b2, g1, be1,
                  g2, be2):
    f64 = np.float64
    # LN gains/shifts fold into the projection weights (exact for any g/be).
    Wq_e = g1.astype(f64)[None, :, None] * Wq.astype(f64)      # [H,D,DK]
    Wk_e = g1.astype(f64)[None, :, None] * Wk.astype(f64)
    Wv_e = g1.astype(f64)[None, :, None] * Wv.astype(f64)
    W1_e = g2.astype(f64)[:, None] * W1.astype(f64)

    def qk_dr(W):   # [H,D,DK] -> [2P, 128, 4(g,X), 2sub, 128(h4*32+dkl)]
        out = np.zeros((2, 128, 4, 2, 128), np.float64)
        for g in range(2):
            for X in range(2):
                for h4 in range(4):
                    h = 4 * g + h4
                    blk = W[h, :, 32 * X:32 * X + 32]      # [D, 32]
                    m = h4 * 32
                    out[:, :, 2 * g + X, :, m:m + 32] = (
                        blk.reshape(2, 2, 128, 32).transpose(0, 2, 1, 3))
        return out

    def v_dr(W):    # [H,D,DK] -> [2, 128, 2, 512], col = h*64+dk
        Wf = np.transpose(W, (1, 0, 2)).reshape(D, H * DK)   # [D, 512]
        return Wf.reshape(2, 2, 128, H * DK).transpose(0, 2, 1, 3)

    out = {}
    out["wq"] = (WSCALE * qk_dr(Wq_e)).astype(E4NP)
    out["wk"] = (WSCALE * qk_dr(Wk_e)).astype(E4NP)
    out["wv"] = (WSCALE * v_dr(Wv_e)).astype(E4NP)
    out["wp"] = (Wp.astype(f64).reshape(H, 64, D) / WSCALE).astype(BFNP)
    out["w1"] = (WSCALE * W1_e.reshape(2, 2, 128, 16, 128)
                 .transpose(0, 2, 3, 1, 4)).astype(E4NP)
    out["w2"] = (WSCALE * W2.astype(f64).reshape(8, 2, 128, D).transpose(0, 2, 1, 3)).astype(E4NP)
    a = np.arange(128)
    out["mask_tri"] = (a[:, None] <= a[None, :]).astype(E4NP)
    return out


def _reference_fallback(x, Wq, bq, Wk, bk, Wv, bv, Wp, bp, W1, b1, W2, b2,
                        g1, be1, g2, be2):
    """Exact numpy fallback for inputs outside the fast path (nonzero biases)."""
    def ln(v, g, b):
        mu = v.mean(-1, keepdims=True)
        var = v.var(-1, keepdims=True)
        return (v - mu) / np.sqrt(var + EPS) * g + b

    h = ln(x, g1, be1)
    q = np.einsum('btd,hdk->bhtk', h, Wq) + bq[:, None, :]
    k = np.einsum('btd,hdk->bhtk', h, Wk) + bk[:, None, :]
    v = np.einsum('btd,hdk->bhtk', h, Wv) + bv[:, None, :]
    s = np.einsum('bhtk,bhsk->bhts', q, k)
    causal = np.tril(np.ones((T, T), dtype=bool))
    s = np.where(causal, s, -1e9)
    s = s / np.sqrt(DK)
    s = s - s.max(-1, keepdims=True)
    p = np.exp(s)
    p = p / p.sum(-1, keepdims=True)
    o = np.einsum('bhts,bhsk->bhtk', p, v)
    o = o.transpose(0, 2, 1, 3).reshape(B, T, D)
    x = x + o @ Wp + bp
    h2 = ln(x, g2, be2)
    ff = np.maximum(h2 @ W1 + b1, 0.0) @ W2 + b2
    return (x + ff).astype(np.float32)


def kernel(x, Wq, bq, Wk, bk, Wv, bv, Wp, bp, W1, b1, W2, b2, g1, be1, g2, be2):
    args = [np.asarray(a) for a in
            (x, Wq, bq, Wk, bk, Wv, bv, Wp, bp, W1, b1, W2, b2, g1, be1, g2, be2)]
    (x, Wq, bq, Wk, bk, Wv, bv, Wp, bp, W1, b1, W2, b2, g1, be1, g2, be2) = args
    x = np.asarray(x, np.float32)

    # Effective biases after LN folding; the device fast path assumes zeros.
    f64 = np.float64
    bq_e = bq.astype(f64) + np.einsum("d,hdk->hk", be1.astype(f64), Wq.astype(f64))
    bk_e = bk.astype(f64) + np.einsum("d,hdk->hk", be1.astype(f64), Wk.astype(f64))
    bv_e = bv.astype(f64) + np.einsum("d,hdk->hk", be1.astype(f64), Wv.astype(f64))
    b1_e = b1.astype(f64) + be2.astype(f64) @ W1.astype(f64)
    if any(np.abs(b).max() > 1e-9 for b in
           (bq_e, bk_e, bv_e, bp.astype(f64), b1_e, b2.astype(f64))):
        return _reference_fallback(x, Wq, bq, Wk, bk, Wv, bv, Wp, bp, W1, b1,
                                   W2, b2, g1, be1, g2, be2)

    folded = _fold_weights(Wq, bq, Wk, bk, Wv, bv, Wp, bp, W1, b1, W2, b2,
                           g1, be1, g2, be2)

    out = np.empty((B, T, D), np.float32)
    for parity in range(2):
        r0, r1 = REGIONS[parity]
        in_maps = []
        for b in range(B):
            x_own = np.concatenate(
                [x[b, r0 * 512:(r0 + 1) * 512], x[b, r1 * 512:(r1 + 1) * 512]])
            m = dict(folded)
            m["x_full"] = np.ascontiguousarray(x[b])
            m["x_own"] = np.ascontiguousarray(x_own)
            in_maps.append(m)
        nc = _get_nc(parity)
        res = run_bass_kernel_spmd(nc, in_maps, list(range(B)))
        for b in range(B):
            o = res.results[b]["out"]
            out[b, r0 * 512:(r0 + 1) * 512] = o[:512]
            out[b, r1 * 512:(r1 + 1) * 512] = o[512:]
    return out

